# revision 29
# baseline (speedup 1.0000x reference)
"""Trainium2 Bass kernel for nn_CrossAttentionBlock (B=4, C=512, H=W=64).

Decomposition across 8 NeuronCores: core = (batch b, query-half h).
All matmuls bf16 (1 cyc/row on PE), bf16 input DMA, conv stages interleaved
into the attention loop behind the ft prefetch so the in-order PE queue
never stalls the exp cadence, per-query-half AllGather (bf16) so the first
exchange hides under the second half's compute.

Each core:
  interleaved: theta/phi = conv1x1(x1), g^T = conv1x1(x0) (PE, bf16)
  main loop (ACT-bound, ~1.1us/iter): fT[keys, queries] = theta^T phi (PE),
       p = exp(fT) (ACT, bf16 out, fT prefetched 2 iterations ahead),
       y_ext[tok, 65] += p_chunk^T [g | 1] accumulated over key chunks (PE)
       -> softmax numerator cols 0..63 and denominator col 64, token-major,
       packed 8 chunks / 2 PSUM banks (single start/stop per zero region).
  per q-half: batched normalize (reciprocal + broadcast mult + g_b add),
       one bounce DMA, pairwise AllGather (the q=0 gather hides under q=1).
  phase 2: W_y = W [view of y] consumed only as per-channel bn stats (AdaIN
       needs only mean/var of W_y); x0 instance stats + 1/(var_c+eps) done
       on DVE during q=1; final out = r * x0 + t, bf16, DMA split over two
       queues.

SPMD uniformity: the key/spatial axis m and the channel axis c are dummy
(contraction/stat) indices, so each core receives inputs permuted so that
"its" queries and "its" output channels come first; the host un-permutes
the output columns. W_w rows are permuted so the two AllGather chunks land
in contiguous yv row blocks.
"""
import numpy as np
import ml_dtypes
from contextlib import ExitStack

import concourse.bass as bass
import concourse.tile as tile
from concourse import mybir
from concourse.bass_utils import run_bass_kernel_spmd

FP32 = mybir.dt.float32
BF16 = mybir.dt.bfloat16
ALU = mybir.AluOpType
ACTF = mybir.ActivationFunctionType

B, C, H, W = 4, 512, 64, 64
N = H * W          # 4096 tokens
C8 = C // 8        # 64 inner channels
NH = N // 2        # 2048 queries per core
OC = C // 2        # 256 output channels per core
EPS = 1e-5

REPLICA_PAIRS = [[0, 1], [2, 3], [4, 5], [6, 7]]

# yv row blocks delivered by the two AllGathers (see _core_inputs W_p perm):
# gather q=0 -> view rows [0:16] u [32:48]; q=1 -> [16:32] u [48:64].
W_ROW_PERM = np.concatenate([
    np.arange(0, 16), np.arange(32, 48),
    np.arange(16, 32), np.arange(48, 64),
])


def _split_excess_waits(nc, max_waits=1, drain_max=1):
    """walrus here rejects instructions carrying more than ~2 sync waits; move
    extras to preceding NoOps on the same engine (semantics preserved: waits
    run before the instruction, engine streams are sequential)."""
    for blk in nc.main_func.blocks:
        insts = blk.instructions
        k = 0
        while k < len(insts):
            inst = insts[k]
            si = inst.sync_info
            cap = drain_max if inst.opcode == "Drain" else max_waits
            if si is not None and si.on_wait and len(si.on_wait) > cap:
                waits = list(si.on_wait)
                keep = waits[-cap:]
                extra = waits[:-cap]
                pos = k
                for j in range(0, len(extra), cap):
                    nop = mybir.InstNoOp(name=f"{inst.name}-wsplit{j}", ins=[], outs=[])
                    nop.engine = inst.engine
                    nop.sync_info = mybir.SyncInfo(
                        on_wait=extra[j : j + cap], on_update=[]
                    )
                    insts.insert(pos, nop)
                    pos += 1
                    k += 1
                inst.sync_info = mybir.SyncInfo(on_wait=keep, on_update=list(si.on_update))
            k += 1


def build_nc():
    nc = bass.Bass()

    x0 = nc.dram_tensor("x0", [C, N], BF16, kind="ExternalInput")
    x1 = nc.dram_tensor("x1", [C, N], BF16, kind="ExternalInput")
    tp_wT = nc.dram_tensor("tp_wT", [C, 128], BF16, kind="ExternalInput")
    tp_b = nc.dram_tensor("tp_b", [128, 1], FP32, kind="ExternalInput")
    g_wT = nc.dram_tensor("g_wT", [C, C8], BF16, kind="ExternalInput")
    g_b_bc = nc.dram_tensor("g_b_bc", [128, 8 * C8], FP32, kind="ExternalInput")
    W_wTh = nc.dram_tensor("W_wTh", [C8, OC], BF16, kind="ExternalInput")
    W_bh = nc.dram_tensor("W_bh", [128, 2], FP32, kind="ExternalInput")
    out = nc.dram_tensor("out", [OC, N], BF16, kind="ExternalOutput")

    y_bounce = nc.dram_tensor("y_bounce", [NH, C8], BF16)
    y_full0 = nc.dram_tensor("y_full0", [NH, C8], BF16)
    y_full1 = nc.dram_tensor("y_full1", [NH, C8], BF16)
    y_fulls = [y_full0, y_full1]

    with tile.TileContext(nc) as tc, ExitStack() as ctx:
        wpool = ctx.enter_context(tc.tile_pool(name="weights", bufs=1))
        big = ctx.enter_context(tc.tile_pool(name="big", bufs=1))

        # ---- persistent big tensors ----
        x0_sb = big.tile([128, 4, N], BF16)      # c-chunk on middle index
        x1_sb = big.tile([128, 4, N], BF16)      # c-chunk on middle index
        theta_sb = big.tile([C8, N], BF16)       # keys, [64, 4096]
        phi_sb = big.tile([C8, NH], BF16)        # queries (own half), [64, 2048]
        g_extT = big.tile([128, 32, C8 + 1], BF16)  # [m-chunk, 65] per chunk
        yv_sb = big.tile([C8, N], BF16)          # gathered y in view-row layout

        # ---- input DMA first (the per-dma_start issue cost on SP serializes
        # all queue pushes, so order = priority), weights right after the
        # first block pair, then the rest of the inputs ----
        x1_r = x1[:].rearrange("(c p) w -> p c w", c=4)
        x0_r = x0[:].rearrange("(c p) w -> p c w", c=4)
        tp_w_sb = wpool.tile([128, 4, 128], BF16)
        g_w_sb = wpool.tile([128, 4, C8], BF16)
        tp_b_sb = wpool.tile([128, 1], FP32)
        g_b_sb = wpool.tile([128, 8, C8], FP32)
        W_w_sb = wpool.tile([C8, OC], BF16)
        W_b_sb = wpool.tile([128, 2], FP32)

        # one sync-engine DMA queue: x1 sb0 first (gates the first exps),
        # then weights, then x1/x0 superblocks interleaved; stage work is
        # emitted late enough to match these arrival times
        nc.sync.dma_start(out=x1_sb[:, :, 0:1024], in_=x1_r[:, :, 0:1024])
        nc.sync.dma_start(out=tp_w_sb[:],
                          in_=tp_wT[:].rearrange("(c p) w -> p c w", c=4))
        nc.sync.dma_start(out=g_w_sb[:],
                          in_=g_wT[:].rearrange("(c p) w -> p c w", c=4))
        nc.sync.dma_start(out=tp_b_sb[:], in_=tp_b[:])
        nc.sync.dma_start(out=g_b_sb[:], in_=g_b_bc[:].rearrange("p (j w) -> p j w", j=8))
        nc.sync.dma_start(out=W_w_sb[:], in_=W_wTh[:])
        nc.sync.dma_start(out=W_b_sb[:], in_=W_bh[:])
        nc.sync.dma_start(out=x0_sb[:, :, 0:1024], in_=x0_r[:, :, 0:1024])
        for blk in range(1, 4):
            cols = slice(blk * 1024, (blk + 1) * 1024)
            nc.sync.dma_start(out=x1_sb[:, :, cols], in_=x1_r[:, :, cols])
            nc.sync.dma_start(out=x0_sb[:, :, cols], in_=x0_r[:, :, cols])

        nc.gpsimd.memset(g_extT[:, :, C8:C8 + 1], 1.0)

        ps_f = ctx.enter_context(tc.tile_pool(name="ps_f", bufs=2, space="PSUM"))
        ps_y = ctx.enter_context(tc.tile_pool(name="ps_y", bufs=1, space="PSUM"))
        ps_sm = ctx.enter_context(tc.tile_pool(name="ps_sm", bufs=2, space="PSUM"))
        ppool = ctx.enter_context(tc.tile_pool(name="pT", bufs=6))
        ystage = ctx.enter_context(tc.tile_pool(name="ystage", bufs=3))

        def stage1_block(blk):
            """theta/phi conv for x1 block blk (512 tokens)."""
            cols = slice(blk * 512, (blk + 1) * 512)
            ptp = ps_sm.tile([128, 512], FP32, tag="sm", name="ptp")
            for c in range(4):
                nc.tensor.matmul(ptp[:], tp_w_sb[:, c, :], x1_sb[:, c, cols],
                                 start=(c == 0), stop=(c == 3))
            nc.vector.tensor_scalar_add(theta_sb[:, cols], ptp[0:C8, :],
                                        tp_b_sb[0:C8, :])
            if blk < 4:
                nc.vector.tensor_scalar_add(phi_sb[:, cols], ptp[C8:128, :],
                                            tp_b_sb[C8:128, :])

        def stage2_chunk(mi):
            """g conv for token chunk mi (128 tokens), transposed layout."""
            pg = ps_sm.tile([128, 512], FP32, tag="sm", name="pg")
            for c in range(4):
                nc.tensor.matmul(pg[:, 0:C8],
                                 x0_sb[:, c, mi * 128:(mi + 1) * 128],
                                 g_w_sb[:, c, :],
                                 start=(c == 0), stop=(c == 3))
            nc.vector.tensor_copy(g_extT[:, mi, 0:C8], pg[:, 0:C8])

        def emit_ft(q, mi):
            ft = ps_f.tile([128, 1024], FP32, tag="ft", name="ft")
            for s in range(2):
                nc.tensor.matmul(
                    ft[:, s * 512:(s + 1) * 512],
                    theta_sb[:, mi * 128:(mi + 1) * 128],
                    phi_sb[:, q * 1024 + s * 512: q * 1024 + (s + 1) * 512],
                    start=True, stop=True)
            return ft

        def q_tail(q, py):
            """normalize (py rows are already token-major), exchange."""
            ybst = ystage.tile([128, 8, C8], BF16, tag="ybst", name="ybst")
            rec = ystage.tile([128, 8], FP32, tag="rec", name="rec")
            nc.vector.reciprocal(rec[:], py[:, :, C8:C8 + 1])
            ynorm = ystage.tile([128, 8, C8], FP32, tag="ynorm", name="ynorm")
            nc.vector.tensor_tensor(ynorm[:], py[:, :, 0:C8],
                                    rec[:].to_broadcast((128, 8, C8)), ALU.mult)
            nc.vector.tensor_tensor(ybst[:], ynorm[:], g_b_sb[:], ALU.add)
            nc.sync.dma_start(
                out=y_bounce[q * 1024:(q + 1) * 1024, :]
                    .rearrange("(j p) w -> p j w", j=8),
                in_=ybst[:])
            nc.gpsimd.collective_compute(
                "AllGather", ALU.bypass,
                replica_groups=REPLICA_PAIRS,
                ins=[y_bounce[q * 1024:(q + 1) * 1024, :]],
                outs=[y_fulls[q][:]],
            )
            nc.sync.dma_start(
                out=yv_sb[q * 32:(q + 1) * 32, :],
                in_=y_fulls[q][:].rearrange("(a b) w -> a (b w)", a=32))

        # blocks 0,1 of stage1 must precede the loop (they feed ft(0)/ft(1));
        # stage2 chunks 0..3 are emitted inside iteration 0, after the first
        # exp, so the PE queue never stalls on the x0 DMA before ft(0)

        # ---- main attention loop, software-pipelined (ft one step ahead),
        # ---- stage work interleaved into q=0
        for blk in range(2):
            stage1_block(blk)

        steps = [(q, mi) for q in range(2) for mi in range(32)]
        py_tiles = {0: None, 1: None}
        ft_queue = [emit_ft(*steps[0]), emit_ft(*steps[1])]
        STAGE1_AT = {5: 2, 6: 3, 13: 4, 14: 5, 21: 6, 22: 7}
        for idx, (q, mi) in enumerate(steps):
            if mi == 0:
                # token-major y accumulator: 8 query chunks of [128, 65],
                # padded to 128-col stride so no chunk crosses a PSUM bank
                py_tiles[q] = ps_y.tile([128, 8, 128], FP32, name="py")
            py = py_tiles[q]
            ft_cur = ft_queue.pop(0)
            pt = ppool.tile([128, 1024], BF16, name="pt")
            nc.scalar.activation(pt[:], ft_cur[:], ACTF.Exp)
            # prefetch ft two steps ahead (emitted after exp so the pool
            # sees exp as the reader of the buffer being recycled): exp
            # never waits on a just-computed ft, even at low PE clock
            if idx + 2 < len(steps):
                ft_queue.append(emit_ft(*steps[idx + 2]))
            # stage work sits after the ft prefetch in the PE queue, so an
            # input-DMA stall here only delays py (absorbed by the pt pool)
            if q == 0:
                if mi in STAGE1_AT:
                    stage1_block(STAGE1_AT[mi])
                if mi == 0:
                    for k in range(4):
                        stage2_chunk(k)
                if mi + 4 < 32:
                    stage2_chunk(mi + 4)
            # the 8 chunks share two PSUM banks (zero regions): start zeroes
            # a whole 2KB bank, so only the first chunk in each bank starts
            # the group and only the last one stops it
            for j in range(8):
                nc.tensor.matmul(
                    py[:, j, 0:C8 + 1],
                    pt[:, j * 128:(j + 1) * 128],
                    g_extT[:, mi, :],
                    start=(mi == 0 and j % 4 == 0),
                    stop=(mi == 31 and j % 4 == 3))
            if mi == 31:
                q_tail(q, py)
            if q == 0 and mi == 31:
                # x0 instance stats + content-side scalars on DVE while
                # q=1 attention runs
                x_agg = big.tile([128, 2, 2], FP32, name="x_agg")
                for oc in range(2):
                    xst = big.tile([128, 8, 6], FP32, name="xst")
                    for mb in range(8):
                        nc.vector.bn_stats(xst[:, mb, :],
                                           x0_sb[:, oc, mb * 512:(mb + 1) * 512])
                    nc.vector.bn_aggr(x_agg[:, oc, :], xst[:])
                vc_b = big.tile([128, 2], FP32, name="vc_b")
                nc.vector.tensor_scalar_add(vc_b[:], x_agg[:, :, 1], EPS)
                rc_b = big.tile([128, 2], FP32, name="rc_b")
                nc.vector.reciprocal(rc_b[:], vc_b[:])

        # ---- phase 2: W_y stats + per-channel affine + output ----
        with tc.tile_pool(name="sc", bufs=1) as sc, \
             tc.tile_pool(name="outp", bufs=4) as outp:
            # style stats: pw tiles pipelined PE -> DVE bn_stats
            w_agg = sc.tile([128, 2, 2], FP32, name="w_agg")
            for oc in range(2):
                wst = sc.tile([128, 8, 6], FP32, tag=f"wst{oc}", name="wst")
                for mb in range(8):
                    cols = slice(mb * 512, (mb + 1) * 512)
                    if mb % 2 == 0:
                        pw = ps_sm.tile([128, 512], FP32, tag="sm", name="pw")
                        pw_ap = pw[:]
                    else:
                        pwt = ps_f.tile([128, 1024], FP32, tag="ft", name="pwt")
                        pw_ap = pwt[:, 0:512]
                    nc.tensor.matmul(pw_ap, W_w_sb[:, oc * 128:(oc + 1) * 128],
                                     yv_sb[:, cols], start=True, stop=True)
                    nc.vector.bn_stats(wst[:, mb, :], pw_ap)
                nc.vector.bn_aggr(w_agg[:, oc, :], wst[:])

            # r = sqrt((var_s + eps) / (var_c + eps)); t = mu_s - r*mu_c
            # (content-side 1/(var_c+eps) was precomputed during q=1)
            vs_b = sc.tile([128, 2], FP32, name="vs_b")
            nc.vector.tensor_scalar_add(vs_b[:], w_agg[:, :, 1], EPS)
            ratio_b = sc.tile([128, 2], FP32, name="ratio_b")
            nc.vector.tensor_mul(ratio_b[:], vs_b[:], rc_b[:])
            rr_b = sc.tile([128, 2], FP32, name="rr_b")
            nc.scalar.sqrt(rr_b[:], ratio_b[:])
            mus_b = sc.tile([128, 2], FP32, name="mus_b")
            nc.vector.tensor_add(mus_b[:], w_agg[:, :, 0], W_b_sb[:])
            rmc_b = sc.tile([128, 2], FP32, name="rmc_b")
            nc.vector.tensor_mul(rmc_b[:], rr_b[:], x_agg[:, :, 0])
            tt_b = sc.tile([128, 2], FP32, name="tt_b")
            nc.vector.tensor_sub(tt_b[:], mus_b[:], rmc_b[:])

            # final affine split across ACT/DVE/GPSIMD so it drains in
            # parallel with the output DMA
            for k in range(8):
                oc, mb = k // 4, k % 4
                cols = slice(mb * 1024, (mb + 1) * 1024)
                ot = outp.tile([128, 1024], BF16, name="ot")
                rr_ap = rr_b[:, oc:oc + 1]
                tt_ap = tt_b[:, oc:oc + 1]
                nc.vector.tensor_scalar(ot[:], x0_sb[:, oc, cols], rr_ap, tt_ap,
                                        ALU.mult, ALU.add)
                nc.sync.dma_start(out=out[oc * 128:(oc + 1) * 128, cols], in_=ot[:])

    _split_excess_waits(nc)
    return nc


_NC_CACHE = None


def _get_nc():
    global _NC_CACHE
    if _NC_CACHE is None:
        _NC_CACHE = build_nc()
    return _NC_CACHE


def _core_inputs(x0f, x1f, tp_wT, tp_b, g_wT, g_b, W_wT, W_b, core):
    b, half = core // 2, core % 2
    x0b, x1b = x0f[b], x1f[b]
    if half == 0:
        x0p = x0b
        x1p = x1b
        g_wp = g_wT
    else:
        # queries-first column permutation; own-channels-first row permutation
        x1p = np.concatenate([x1b[:, NH:], x1b[:, :NH]], axis=1)
        x0r = np.concatenate([x0b[OC:], x0b[:OC]], axis=0)
        x0p = np.concatenate([x0r[:, NH:], x0r[:, :NH]], axis=1)
        g_wp = np.concatenate([g_wT[OC:], g_wT[:OC]], axis=0)
    # W rows permuted so each AllGather's rows are a contiguous yv block
    W_p = W_wT[W_ROW_PERM][:, half * OC:(half + 1) * OC]
    return {
        "x0": np.ascontiguousarray(x0p.astype(ml_dtypes.bfloat16)),
        "x1": np.ascontiguousarray(x1p.astype(ml_dtypes.bfloat16)),
        "tp_wT": tp_wT,
        "tp_b": tp_b,
        "g_wT": np.ascontiguousarray(g_wp.astype(ml_dtypes.bfloat16)),
        "g_b_bc": np.ascontiguousarray(
            np.broadcast_to(np.tile(g_b, 8), (128, 8 * C8)).astype(np.float32)),
        "W_wTh": np.ascontiguousarray(W_p.astype(ml_dtypes.bfloat16)),
        "W_bh": np.ascontiguousarray(
            W_b[half * OC:(half + 1) * OC].reshape(2, 128).T.astype(np.float32)),
    }


def _make_in_maps(inputs):
    x0 = np.asarray(inputs["x0"], dtype=np.float32)
    x1 = np.asarray(inputs["x1"], dtype=np.float32)
    x0f = x0.reshape(B, C, N)
    x1f = x1.reshape(B, C, N)
    tp_wT = np.ascontiguousarray(
        np.concatenate([np.asarray(inputs["theta_w"], np.float32),
                        np.asarray(inputs["phi_w"], np.float32)], axis=0).T
        .astype(ml_dtypes.bfloat16))
    tp_b = np.ascontiguousarray(
        np.concatenate([np.asarray(inputs["theta_b"], np.float32),
                        np.asarray(inputs["phi_b"], np.float32)])[:, None])
    g_wT = np.ascontiguousarray(np.asarray(inputs["g_w"], np.float32).T)
    W_wT = np.ascontiguousarray(np.asarray(inputs["W_w"], np.float32).T)
    g_b = np.asarray(inputs["g_b"], np.float32)
    W_b = np.asarray(inputs["W_b"], np.float32)
    return [
        _core_inputs(x0f, x1f, tp_wT, tp_b, g_wT, g_b, W_wT, W_b, core)
        for core in range(8)
    ]


def kernel(x0, x1, g_w, g_b, theta_w, theta_b, phi_w, phi_b, W_w, W_b):
    in_maps = _make_in_maps(dict(
        x0=x0, x1=x1, g_w=g_w, g_b=g_b, theta_w=theta_w, theta_b=theta_b,
        phi_w=phi_w, phi_b=phi_b, W_w=W_w, W_b=W_b))
    nc = _get_nc()
    res = run_bass_kernel_spmd(nc, in_maps, core_ids=list(range(8)))

    out = np.empty((B, C, N), dtype=np.float32)
    for core in range(8):
        b, half = core // 2, core % 2
        o = np.asarray(res.results[core]["out"]).astype(np.float32)
        if half == 1:
            o = np.concatenate([o[:, NH:], o[:, :NH]], axis=1)
        out[b, half * OC:(half + 1) * OC] = o
    return out.reshape(B, C, H, W)
